# revision 15
# baseline (speedup 1.0000x reference)
"""Multi-head attention (16 heads, RoPE, causal) on 8 Trainium2 NeuronCores.

Sharding: batch*heads across cores. Core c handles batch b = c//4 and heads
4*(c%4) .. 4*(c%4)+3 (column-split W_q/W_k/W_v, row-split W_o; partial
outputs summed on host).

On-device layout choices:
  - q,k produced transposed [d_head, s] so scores can be computed as
    scoresT [k, q] = kT.T @ qT (contraction over d on partitions).
  - softmax runs over the partition (k) dim: exp on ACT straight out of
    PSUM; the denominator comes free from a ones-column appended to V in
    the attn@V matmul (outT[64] = sum_k exp).
  - causal structure: for query tile j (512 wide) only k-chunks <= 4j+3
    are computed; the diagonal 4-chunk group is masked with a
    host-precomputed 0/1 tile; everything above the diagonal is skipped
    (host leaves zeros).
  - attn is written to DRAM packed/transposed; the host scatters it into
    the [B,H,S,S] output (pure data movement, no math).
Matmuls use float32r (full fp32 storage, single-pass PE) except the
exactness-sensitive broadcast outer-product which stays fp32.
"""

import os
import numpy as np

import concourse.bass as bass
import concourse.bacc as bacc
import concourse.mybir as mybir
import concourse.tile as tile
from concourse import bass_utils

F32 = mybir.dt.float32
F32R = mybir.dt.float32r

B, S, D, H, DK = 2, 2048, 1024, 16, 64
NCORES = 8
GRP = NCORES // B          # core groups per batch
HL = H // GRP              # heads per core
DL = HL * DK               # local projected dim
NJ = S // 512              # query tiles per head
NKC = S // 128             # k chunks
NDC = D // 128             # contraction chunks for projections
TRI = [0, 1, 3, 6]         # sum_{i<j}(i+1)
NGRP_HEAD = TRI[NJ - 1] + NJ          # kept 4-chunk groups per head (10)
NBLK = HL * NGRP_HEAD                 # kept groups per core (40)
SWAP_MASK = [i ^ 1 for i in range(32)]

_NC_CACHE = None
LAST_RESULTS = None  # BassKernelResults of the most recent kernel() call


def _r(ap):
    return ap.bitcast(F32R)


def _build_nc():
    nc = bacc.Bacc("TRN2", target_bir_lowering=False, debug=False,
                   enable_asserts=False, num_devices=NCORES)

    xt_q = nc.dram_tensor("xt_q", [D, S], F32R, kind="ExternalInput")
    xt_k = nc.dram_tensor("xt_k", [D, S], F32R, kind="ExternalInput")
    xt_v = nc.dram_tensor("xt_v", [D, S], F32R, kind="ExternalInput")
    w_q = nc.dram_tensor("w_qT", [D, DL], F32R, kind="ExternalInput")
    w_k = nc.dram_tensor("w_kT", [D, DL], F32R, kind="ExternalInput")
    w_v = nc.dram_tensor("w_vT", [D, DL], F32R, kind="ExternalInput")
    w_o = nc.dram_tensor("w_oT", [DL, D], F32R, kind="ExternalInput")
    cos_d = nc.dram_tensor("cos_t", [128, S], F32, kind="ExternalInput")
    sin_d = nc.dram_tensor("sin_t", [128, S], F32, kind="ExternalInput")
    bq_d = nc.dram_tensor("bq_s", [128, 2], F32, kind="ExternalInput")
    bk_d = nc.dram_tensor("bk_s", [128, 2], F32, kind="ExternalInput")
    mask_d = nc.dram_tensor("mask_diag", [128, 4 * NJ, 512], F32,
                            kind="ExternalInput")
    attn_o = nc.dram_tensor("attn_out", [128, 4 * NBLK, 512], F32,
                            kind="ExternalOutput")
    out_o = nc.dram_tensor("out_part", [S // 128, 128, D], F32,
                           kind="ExternalOutput")

    with tile.TileContext(nc) as tc:
        with (
            tc.tile_pool(name="cpool", bufs=1) as cpool,
            tc.tile_pool(name="c2pool", bufs=1) as c2pool,
        ):
            wo_sb = cpool.tile([128, 2, D], F32R, tag="wo")
            bq_sb = cpool.tile([128, 2], F32, tag="bq")
            nc.sync.dma_start(bq_sb[:], bq_d.ap())
            bk_sb = cpool.tile([128, 2], F32, tag="bk")
            nc.sync.dma_start(bk_sb[:], bk_d.ap())
            ones_sb = cpool.tile([128, 128], F32, tag="ones")
            nc.vector.memset(ones_sb[:], 1.0)

            qT = c2pool.tile([128, 2, S], F32R, tag="qT")
            kT = c2pool.tile([128, 2, S], F32R, tag="kT")
            vx = c2pool.tile([128, NKC, HL, 66], F32R, tag="vx")
            yT = c2pool.tile([128, 2, S], F32R, tag="yT")
            nc.vector.tensor_copy(
                vx[:, :, :, 64:65],
                ones_sb[:, 0:1][:, :, None, None].to_broadcast(
                    (128, NKC, HL, 1)))

            # ---------------- phase 1: projections + rope ----------------
            with (
                tc.tile_pool(name="p1w", bufs=1) as p1w,
                tc.tile_pool(name="p1x", bufs=3) as p1x,
                tc.tile_pool(name="p1r", bufs=2) as p1r,
                tc.tile_pool(name="p1ps", bufs=3, space="PSUM") as p1ps,
            ):
                wq_sb = p1w.tile([128, NDC, DL], F32R, tag="wq")
                nc.sync.dma_start(
                    wq_sb[:], w_q.ap().rearrange("(dc p) o -> p dc o", p=128))
                wk_sb = p1w.tile([128, NDC, DL], F32R, tag="wk")
                wv_sb = p1w.tile([128, NDC, DL], F32R, tag="wv")
                cos_sb = p1w.tile([128, S], F32, tag="cos")
                sin_sb = p1w.tile([128, S], F32, tag="sin")

                for st in range(S // 512):
                    scol = slice(512 * st, 512 * (st + 1))

                    for name, xdram, wsb, bsb, dst in (
                        ("q", xt_q, wq_sb, bq_sb, qT),
                        ("k", xt_k, wk_sb, bk_sb, kT),
                    ):
                        xsb = p1x.tile([128, NDC, 512], F32R, tag="xt")
                        nc.sync.dma_start(
                            xsb[:],
                            xdram.ap()[:, scol].rearrange(
                                "(dc p) s -> p dc s", p=128))
                        if st == 0 and name == "q":
                            nc.sync.dma_start(cos_sb[:], cos_d.ap())
                            nc.sync.dma_start(sin_sb[:], sin_d.ap())
                        if st == 0 and name == "k":
                            nc.sync.dma_start(
                                wk_sb[:],
                                w_k.ap().rearrange("(dc p) o -> p dc o",
                                                   p=128))
                        for dt in range(2):
                            ps = p1ps.tile([128, 512], F32, tag="qk")
                            for dc in range(NDC):
                                nc.tensor.matmul(
                                    ps[:],
                                    wsb[:, dc, 128 * dt:128 * (dt + 1)],
                                    xsb[:, dc, :],
                                    start=(dc == 0), stop=(dc == NDC - 1))
                            raw = p1r.tile([128, 512], F32, tag="raw")
                            nc.vector.tensor_scalar_add(
                                raw[:], ps[:], bsb[:, dt:dt + 1])
                            swp = p1r.tile([128, 512], F32, tag="swp")
                            nc.vector.stream_shuffle(swp[:], raw[:], SWAP_MASK)
                            tmp = p1r.tile([128, 512], F32, tag="tmp")
                            nc.vector.tensor_mul(
                                out=tmp[:], in0=swp[:], in1=sin_sb[:, scol])
                            tmp2 = p1r.tile([128, 512], F32, tag="tmp2")
                            nc.vector.tensor_mul(
                                out=tmp2[:], in0=raw[:], in1=cos_sb[:, scol])
                            nc.vector.tensor_add(
                                out=dst[:, dt, scol], in0=tmp2[:],
                                in1=tmp[:])

                    xsb = p1x.tile([128, NDC, 512], F32R, tag="xt")
                    nc.sync.dma_start(
                        xsb[:],
                        xt_v.ap()[:, scol].rearrange("(dc p) s -> p dc s",
                                                     p=128))
                    if st == 0:
                        nc.sync.dma_start(
                            wv_sb[:],
                            w_v.ap().rearrange("(dc p) o -> p dc o", p=128))
                    for si in range(4):
                        sc = 4 * st + si
                        ps = p1ps.tile([128, DL], F32, tag="v")
                        for dc in range(NDC):
                            nc.tensor.matmul(
                                ps[:],
                                xsb[:, dc, 128 * si:128 * (si + 1)],
                                wv_sb[:, dc, :],
                                start=(dc == 0), stop=(dc == NDC - 1))
                        nc.scalar.copy(
                            vx[:, sc, :, 0:64],
                            ps.rearrange("p (h d) -> p h d", d=DK))

            # ------------- phase 2+3: attention + output projection -------------
            with (
                tc.tile_pool(name="p2m", bufs=2) as p2m,
                tc.tile_pool(name="p2e", bufs=7) as p2e,
                tc.tile_pool(name="p2a", bufs=2) as p2a,
                tc.tile_pool(name="p2s", bufs=2) as p2s,
                tc.tile_pool(name="p3", bufs=2) as p3,
                tc.tile_pool(name="p2ps", bufs=2, space="PSUM") as p2ps,
                tc.tile_pool(name="p2sc", bufs=2, space="PSUM") as p2sc,
                tc.tile_pool(name="p3ps", bufs=2, space="PSUM") as p3ps,
            ):
                nc.sync.dma_start(
                    wo_sb[:], w_o.ap().rearrange("(jc p) i -> p jc i", p=128))

                masks = {}

                def emit_kloop(h, j):
                    hp, hc = h % 2, h // 2
                    prow = slice(64 * hp, 64 * hp + 64)
                    qcol = slice(512 * j, 512 * (j + 1))
                    pv = p2ps.tile([128, 512], F32, tag="pv")
                    if h == 0:
                        m = p2m.tile([128, 4, 512], F32, tag="mask")
                        nc.sync.dma_start(
                            m[:], mask_d.ap()[:, 4 * j:4 * j + 4, :])
                        masks[j] = m
                    mask_sb = masks[j]
                    ecols = []
                    for g in range(j + 1):
                        ecol = p2e.tile([128, 4, 512], F32R, tag="ecol")
                        ecols.append(ecol)
                        for u in range(2):
                            sc_ps = p2sc.tile([128, 2, 512], F32, tag="sc")
                            for w in range(2):
                                kc = 4 * g + 2 * u + w
                                nc.tensor.matmul(
                                    sc_ps[:, w, :],
                                    kT[prow, hc, 128 * kc:128 * (kc + 1)],
                                    qT[prow, hc, qcol],
                                    start=True, stop=True)
                            nc.scalar.activation(
                                ecol[:, 2 * u:2 * u + 2, :], sc_ps[:],
                                mybir.ActivationFunctionType.Exp,
                                scale=float(1.0 / np.sqrt(DK)))
                        if g == j:
                            nc.vector.tensor_mul(
                                out=ecol[:],
                                in0=ecol[:].bitcast(F32),
                                in1=mask_sb[:])
                        for t in range(4):
                            kc = 4 * g + t
                            nc.tensor.matmul(
                                pv[0:65, :],
                                vx[:, kc, h, 0:65],
                                ecol[:, t, :],
                                start=(g == 0 and t == 0),
                                stop=(g == j and t == 3),
                                skip_group_check=True)
                    return pv, ecols

                def emit_epilogue(h, j, pv, ecols):
                    hp, hc = h % 2, h // 2
                    qcol = slice(512 * j, 512 * (j + 1))
                    # denominators: psum row 64 -> sbuf -> 4 partitions for a
                    # wide reciprocal -> back to a [1, 512] row -> broadcast.
                    drow = p2s.tile([128, 512], F32, tag="drow")
                    nc.scalar.copy(drow[64:65, :], pv[64:65, :])
                    dcol = p2s.tile([4, 128], F32, tag="dcol")
                    nc.gpsimd.dma_start(dcol[:], drow[64:65, :])
                    rcol = p2s.tile([4, 128], F32, tag="rcol")
                    nc.vector.reciprocal(rcol[:], dcol[:])
                    rrow = p2s.tile([1, 512], F32, tag="rrow")
                    nc.gpsimd.dma_start(rrow[:], rcol[:])
                    bc = p2s.tile([128, 512], F32, tag="bcs")
                    nc.gpsimd.partition_broadcast(bc[:], rrow[:])
                    # normalize + write attn
                    for g in range(j + 1):
                        gi = NGRP_HEAD * h + TRI[j] + g
                        ecol = ecols[g]
                        ast = p2a.tile([128, 4, 512], F32, tag="ast")
                        eng = nc.gpsimd if (g % 3 == 1) else nc.vector
                        eng.tensor_mul(
                            out=ast[:],
                            in0=ecol[:].bitcast(F32),
                            in1=bc[:, None, :].to_broadcast((128, 4, 512)))
                        nc.sync.dma_start(
                            attn_o.ap()[:, 4 * gi:4 * gi + 4, :], ast[:])
                    # y^T (normalized attn @ v, transposed)
                    if hp == 0:
                        nc.vector.tensor_mul(
                            out=yT[0:64, hc, qcol], in0=pv[0:64, :],
                            in1=bc[0:64, :])
                    else:
                        yst = p2s.tile([128, 512], F32R, tag="yst")
                        nc.vector.tensor_mul(
                            out=yst[0:64, :], in0=pv[0:64, :],
                            in1=bc[0:64, :])
                        nc.gpsimd.dma_start(yT[64:128, hc, qcol],
                                            yst[0:64, :])

                def emit_outproj(j):
                    for sc in range(4 * j, 4 * j + 4):
                        ost = p3.tile([128, D], F32, tag="ost")
                        for it in range(2):
                            ps = p3ps.tile([128, 512], F32, tag="op")
                            for jc in range(2):
                                nc.tensor.matmul(
                                    ps[:],
                                    yT[:, jc, 128 * sc:128 * (sc + 1)],
                                    wo_sb[:, jc, 512 * it:512 * (it + 1)],
                                    start=(jc == 0), stop=(jc == 1))
                            nc.scalar.copy(
                                ost[:, 512 * it:512 * (it + 1)], ps[:])
                        nc.sync.dma_start(out_o.ap()[sc], ost[:])

                pending = None
                pending_oproj = None
                for j in range(NJ):
                    for h in range(HL):
                        args = emit_kloop(h, j)
                        if pending is not None:
                            emit_epilogue(*pending)
                            if pending[0] == HL - 1:
                                pending_oproj = pending[1]
                        elif pending_oproj is not None:
                            pass
                        if pending_oproj is not None and h == 1:
                            emit_outproj(pending_oproj)
                            pending_oproj = None
                        pending = (h, j) + args
                emit_epilogue(*pending)
                emit_outproj(NJ - 1)

    nc.compile()
    return nc


def _get_nc():
    global _NC_CACHE
    if _NC_CACHE is None:
        _NC_CACHE = _build_nc()
    return _NC_CACHE


def _rope_tables():
    inv = (np.float32(1.0) /
           (np.float32(10000.0) **
            (np.arange(0, DK, 2, dtype=np.float32) / np.float32(DK))))
    pos = np.arange(S, dtype=np.float32)
    ang = pos[:, None] * inv[None, :]                    # [S, DK/2] f32
    sin = np.repeat(np.sin(ang), 2, axis=-1)             # [S, DK]
    cos = np.repeat(np.cos(ang), 2, axis=-1)
    sgn = np.where(np.arange(DK) % 2 == 0, np.float32(-1), np.float32(1))
    cosT = np.ascontiguousarray(np.tile(cos.T, (2, 1)), dtype=np.float32)
    ssT = np.ascontiguousarray(np.tile((sin * sgn).T, (2, 1)),
                               dtype=np.float32)
    return cosT, ssT  # each [128, S]


def _numpy_reference(Q, K, V, Wq, bq, Wk, bk, Wv, bv, Wo, bo, mask):
    """Fallback for non-causal masks (never hit with the standard inputs)."""
    def proj(x, W, b):
        return (x @ W.T + b).reshape(B, S, H, DK).transpose(0, 2, 1, 3)
    q, k, v = proj(Q, Wq, bq), proj(K, Wk, bk), proj(V, Wv, bv)
    inv = (np.float32(1.0) /
           (np.float32(10000.0) **
            (np.arange(0, DK, 2, dtype=np.float32) / np.float32(DK))))
    ang = np.arange(S, dtype=np.float32)[:, None] * inv[None, :]
    sin_u = np.repeat(np.sin(ang), 2, -1)
    cos_u = np.repeat(np.cos(ang), 2, -1)

    def rot(x):
        x1, x2 = x[..., 0::2], x[..., 1::2]
        return np.stack((-x2, x1), -1).reshape(x.shape)
    q = q * cos_u[None, None] + rot(q) * sin_u[None, None]
    k = k * cos_u[None, None] + rot(k) * sin_u[None, None]
    sc = np.einsum('bhqd,bhkd->bhqk', q, k) / np.sqrt(np.float32(DK))
    sc = np.where(mask == 0, -np.inf, sc)
    sc = sc - sc.max(-1, keepdims=True)
    e = np.exp(sc)
    attn = e / e.sum(-1, keepdims=True)
    out = np.einsum('bhqk,bhkd->bhqd', attn, v)
    out = out.transpose(0, 2, 1, 3).reshape(B, S, D) @ Wo.T + bo
    return out.astype(np.float32), attn.astype(np.float32)


def kernel(Q, K, V, Wq, bq, Wk, bk, Wv, bv, Wo, bo, mask):
    global LAST_RESULTS
    Q = np.asarray(Q, np.float32)
    K = np.asarray(K, np.float32)
    V = np.asarray(V, np.float32)
    Wq, Wk, Wv, Wo = (np.asarray(a, np.float32) for a in (Wq, Wk, Wv, Wo))
    bq, bk, bv, bo = (np.asarray(a, np.float32) for a in (bq, bk, bv, bo))
    mask = np.asarray(mask)

    causal = bool(
        (mask == np.tril(np.ones((S, S), mask.dtype))[None, None]).all())
    if not causal:
        return _numpy_reference(Q, K, V, Wq, bq, Wk, bk, Wv, bv, Wo, bo, mask)

    nc = _get_nc()
    cosT, ssT = _rope_tables()

    # diagonal-block 0/1 mask tiles in [k, q] layout: tile (j, t) covers
    # k in [512j+128t, +128), q in [512j, +512)
    mdiag = np.zeros((128, 4 * NJ, 512), np.float32)
    for j in range(NJ):
        for t in range(4):
            k0 = 512 * j + 128 * t
            kk = np.arange(k0, k0 + 128)
            qq = np.arange(512 * j, 512 * (j + 1))
            mdiag[:, 4 * j + t, :] = (kk[:, None] <= qq[None, :])

    in_maps = []
    for c in range(NCORES):
        b, g = c // GRP, c % GRP
        sl = slice(DL * g, DL * (g + 1))
        in_maps.append({
            "xt_q": np.ascontiguousarray(Q[b].T),
            "xt_k": np.ascontiguousarray(K[b].T),
            "xt_v": np.ascontiguousarray(V[b].T),
            "w_qT": np.ascontiguousarray(Wq[sl].T),
            "w_kT": np.ascontiguousarray(Wk[sl].T),
            "w_vT": np.ascontiguousarray(Wv[sl].T),
            "w_oT": np.ascontiguousarray(Wo[:, sl].T),
            "cos_t": cosT,
            "sin_t": ssT,
            "bq_s": np.ascontiguousarray(bq[sl].reshape(2, 128).T),
            "bk_s": np.ascontiguousarray(bk[sl].reshape(2, 128).T),
            "mask_diag": mdiag,
        })

    trace = bool(int(os.environ.get("KERNEL_TRACE", "0")))
    res = bass_utils.run_bass_kernel_spmd(
        nc, in_maps, core_ids=list(range(NCORES)), trace=trace)
    LAST_RESULTS = res

    attn = np.zeros((B, H, S, S), np.float32)
    out = np.zeros((B, S, D), np.float32)
    for c in range(NCORES):
        b, g = c // GRP, c % GRP
        chunk = res.results[c]["attn_out"].reshape(128, NBLK, 4, 512)
        for h in range(HL):
            for j in range(NJ):
                for gg in range(j + 1):
                    gi = NGRP_HEAD * h + TRI[j] + gg
                    blk = chunk[:, gi]  # [p, t, q]
                    attn[b, HL * g + h, 512 * j:512 * (j + 1),
                         512 * gg:512 * (gg + 1)] = (
                        blk.transpose(2, 1, 0).reshape(512, 512))
        out[b] += res.results[c]["out_part"].reshape(S, D)
    out += (bv @ Wo.T + bo)[None, None, :]
    return out, attn


# revision 16
# speedup vs baseline: 1.3385x; 1.3385x over previous
"""Multi-head attention (16 heads, RoPE, causal) on 8 Trainium2 NeuronCores.

Sharding: batch*heads across cores. Core c handles batch b = c//4 and heads
4*(c%4) .. 4*(c%4)+3 (column-split W_q/W_k/W_v, row-split W_o; partial
outputs summed on host).

On-device layout choices:
  - q,k produced transposed [d_head, s] so scores can be computed as
    scoresT [k, q] = kT.T @ qT (contraction over d on partitions).
  - softmax runs over the partition (k) dim: exp on ACT straight out of
    PSUM; the denominator comes free from a ones-column appended to V in
    the attn@V matmul (outT[64] = sum_k exp).
  - causal structure: for query tile j (512 wide) only k-chunks <= 4j+3
    are computed; the diagonal 4-chunk group is masked with a
    host-precomputed 0/1 tile; everything above the diagonal is skipped
    (host leaves zeros).
  - attn is written to DRAM packed/transposed; the host scatters it into
    the [B,H,S,S] output (pure data movement, no math).
Matmuls use float32r (full fp32 storage, single-pass PE) except the
exactness-sensitive broadcast outer-product which stays fp32.
"""

import os
import numpy as np

import concourse.bass as bass
import concourse.bacc as bacc
import concourse.mybir as mybir
import concourse.tile as tile
from concourse import bass_utils

F32 = mybir.dt.float32
F32R = mybir.dt.float32r

B, S, D, H, DK = 2, 2048, 1024, 16, 64
NCORES = 8
GRP = NCORES // B          # core groups per batch
HL = H // GRP              # heads per core
DL = HL * DK               # local projected dim
NJ = S // 512              # query tiles per head
NKC = S // 128             # k chunks
NDC = D // 128             # contraction chunks for projections
TRI = [0, 1, 3, 6]         # sum_{i<j}(i+1)
NGRP_HEAD = TRI[NJ - 1] + NJ          # kept 4-chunk groups per head (10)
NBLK = HL * NGRP_HEAD                 # kept groups per core (40)
SWAP_MASK = [i ^ 1 for i in range(32)]

_NC_CACHE = None
LAST_RESULTS = None  # BassKernelResults of the most recent kernel() call


def _r(ap):
    return ap.bitcast(F32R)


def _build_nc():
    nc = bacc.Bacc("TRN2", target_bir_lowering=False, debug=False,
                   enable_asserts=False, num_devices=NCORES)

    xt_q = nc.dram_tensor("xt_q", [D, S], F32R, kind="ExternalInput")
    xt_k = nc.dram_tensor("xt_k", [D, S], F32R, kind="ExternalInput")
    xt_v = nc.dram_tensor("xt_v", [D, S], F32R, kind="ExternalInput")
    w_q = nc.dram_tensor("w_qT", [D, DL], F32R, kind="ExternalInput")
    w_k = nc.dram_tensor("w_kT", [D, DL], F32R, kind="ExternalInput")
    w_v = nc.dram_tensor("w_vT", [D, DL], F32R, kind="ExternalInput")
    w_o = nc.dram_tensor("w_oT", [DL, D], F32R, kind="ExternalInput")
    cos_d = nc.dram_tensor("cos_t", [128, S], F32, kind="ExternalInput")
    sin_d = nc.dram_tensor("sin_t", [128, S], F32, kind="ExternalInput")
    bq_d = nc.dram_tensor("bq_s", [128, 2], F32, kind="ExternalInput")
    bk_d = nc.dram_tensor("bk_s", [128, 2], F32, kind="ExternalInput")
    mask_d = nc.dram_tensor("mask_diag", [128, 4 * NJ, 512], F32,
                            kind="ExternalInput")
    attn_o = nc.dram_tensor("attn_out", [128, 4 * NBLK, 512], F32,
                            kind="ExternalOutput")
    out_o = nc.dram_tensor("out_part", [S // 128, 128, D], F32,
                           kind="ExternalOutput")

    with tile.TileContext(nc) as tc:
        with (
            tc.tile_pool(name="cpool", bufs=1) as cpool,
            tc.tile_pool(name="c2pool", bufs=1) as c2pool,
        ):
            wo_sb = cpool.tile([128, 2, D], F32R, tag="wo")
            bq_sb = cpool.tile([128, 2], F32, tag="bq")
            nc.sync.dma_start(bq_sb[:], bq_d.ap())
            bk_sb = cpool.tile([128, 2], F32, tag="bk")
            nc.sync.dma_start(bk_sb[:], bk_d.ap())
            ones_sb = cpool.tile([128, 128], F32, tag="ones")
            nc.vector.memset(ones_sb[:], 1.0)

            qT = c2pool.tile([128, 2, S], F32R, tag="qT")
            kT = c2pool.tile([128, 2, S], F32R, tag="kT")
            vx = c2pool.tile([128, NKC, HL, 66], F32R, tag="vx")
            yT = c2pool.tile([128, 2, S], F32R, tag="yT")
            nc.vector.tensor_copy(
                vx[:, :, :, 64:65],
                ones_sb[:, 0:1][:, :, None, None].to_broadcast(
                    (128, NKC, HL, 1)))

            # ---------------- phase 1: projections + rope ----------------
            with (
                tc.tile_pool(name="p1w", bufs=1) as p1w,
                tc.tile_pool(name="p1x", bufs=3) as p1x,
                tc.tile_pool(name="p1r", bufs=2) as p1r,
                tc.tile_pool(name="p1ps", bufs=3, space="PSUM") as p1ps,
            ):
                wq_sb = p1w.tile([128, NDC, DL], F32R, tag="wq")
                nc.sync.dma_start(
                    wq_sb[:], w_q.ap().rearrange("(dc p) o -> p dc o", p=128))
                wk_sb = p1w.tile([128, NDC, DL], F32R, tag="wk")
                wv_sb = p1w.tile([128, NDC, DL], F32R, tag="wv")
                cos_sb = p1w.tile([128, S], F32, tag="cos")
                sin_sb = p1w.tile([128, S], F32, tag="sin")

                for st in range(S // 512):
                    scol = slice(512 * st, 512 * (st + 1))

                    for name, xdram, wsb, bsb, dst in (
                        ("q", xt_q, wq_sb, bq_sb, qT),
                        ("k", xt_k, wk_sb, bk_sb, kT),
                    ):
                        xsb = p1x.tile([128, NDC, 512], F32R, tag="xt")
                        nc.sync.dma_start(
                            xsb[:],
                            xdram.ap()[:, scol].rearrange(
                                "(dc p) s -> p dc s", p=128))
                        if st == 0 and name == "q":
                            nc.sync.dma_start(cos_sb[:], cos_d.ap())
                            nc.sync.dma_start(sin_sb[:], sin_d.ap())
                        if st == 0 and name == "k":
                            nc.sync.dma_start(
                                wk_sb[:],
                                w_k.ap().rearrange("(dc p) o -> p dc o",
                                                   p=128))
                        for dt in range(2):
                            ps = p1ps.tile([128, 512], F32, tag="qk")
                            for dc in range(NDC):
                                nc.tensor.matmul(
                                    ps[:],
                                    wsb[:, dc, 128 * dt:128 * (dt + 1)],
                                    xsb[:, dc, :],
                                    start=(dc == 0), stop=(dc == NDC - 1))
                            raw = p1r.tile([128, 512], F32, tag="raw")
                            nc.vector.tensor_scalar_add(
                                raw[:], ps[:], bsb[:, dt:dt + 1])
                            swp = p1r.tile([128, 512], F32, tag="swp")
                            nc.vector.stream_shuffle(swp[:], raw[:], SWAP_MASK)
                            tmp = p1r.tile([128, 512], F32, tag="tmp")
                            nc.vector.tensor_mul(
                                out=tmp[:], in0=swp[:], in1=sin_sb[:, scol])
                            tmp2 = p1r.tile([128, 512], F32, tag="tmp2")
                            nc.vector.tensor_mul(
                                out=tmp2[:], in0=raw[:], in1=cos_sb[:, scol])
                            nc.vector.tensor_add(
                                out=dst[:, dt, scol], in0=tmp2[:],
                                in1=tmp[:])

                    xsb = p1x.tile([128, NDC, 512], F32R, tag="xt")
                    nc.sync.dma_start(
                        xsb[:],
                        xt_v.ap()[:, scol].rearrange("(dc p) s -> p dc s",
                                                     p=128))
                    if st == 0:
                        nc.sync.dma_start(
                            wv_sb[:],
                            w_v.ap().rearrange("(dc p) o -> p dc o", p=128))
                    for si in range(4):
                        sc = 4 * st + si
                        ps = p1ps.tile([128, DL], F32, tag="v")
                        for dc in range(NDC):
                            nc.tensor.matmul(
                                ps[:],
                                xsb[:, dc, 128 * si:128 * (si + 1)],
                                wv_sb[:, dc, :],
                                start=(dc == 0), stop=(dc == NDC - 1))
                        nc.scalar.copy(
                            vx[:, sc, :, 0:64],
                            ps.rearrange("p (h d) -> p h d", d=DK))

            # ------------- phase 2+3: attention + output projection -------------
            with (
                tc.tile_pool(name="p2m", bufs=2) as p2m,
                tc.tile_pool(name="p2e", bufs=7) as p2e,
                tc.tile_pool(name="p2a", bufs=2) as p2a,
                tc.tile_pool(name="p2s", bufs=2) as p2s,
                tc.tile_pool(name="p3", bufs=2) as p3,
                tc.tile_pool(name="p2ps", bufs=2, space="PSUM") as p2ps,
                tc.tile_pool(name="p2sc", bufs=2, space="PSUM") as p2sc,
                tc.tile_pool(name="p3ps", bufs=2, space="PSUM") as p3ps,
            ):
                nc.sync.dma_start(
                    wo_sb[:], w_o.ap().rearrange("(jc p) i -> p jc i", p=128))

                masks = {}

                def emit_kloop(h, j):
                    hp, hc = h % 2, h // 2
                    prow = slice(64 * hp, 64 * hp + 64)
                    qcol = slice(512 * j, 512 * (j + 1))
                    pv = p2ps.tile([128, 512], F32, tag="pv")
                    if h == 0:
                        m = p2m.tile([128, 4, 512], F32, tag="mask")
                        nc.sync.dma_start(
                            m[:], mask_d.ap()[:, 4 * j:4 * j + 4, :])
                        masks[j] = m
                    mask_sb = masks[j]
                    ecols = []
                    for g in range(j + 1):
                        ecol = p2e.tile([128, 4, 512], F32R, tag="ecol")
                        ecols.append(ecol)
                        for u in range(2):
                            sc_ps = p2sc.tile([128, 2, 512], F32, tag="sc")
                            for w in range(2):
                                kc = 4 * g + 2 * u + w
                                nc.tensor.matmul(
                                    sc_ps[:, w, :],
                                    kT[prow, hc, 128 * kc:128 * (kc + 1)],
                                    qT[prow, hc, qcol],
                                    start=True, stop=True)
                            nc.scalar.activation(
                                ecol[:, 2 * u:2 * u + 2, :], sc_ps[:],
                                mybir.ActivationFunctionType.Exp,
                                scale=float(1.0 / np.sqrt(DK)))
                        if g == j:
                            nc.vector.tensor_mul(
                                out=ecol[:],
                                in0=ecol[:].bitcast(F32),
                                in1=mask_sb[:])
                        for t in range(4):
                            kc = 4 * g + t
                            nc.tensor.matmul(
                                pv[0:65, :],
                                vx[:, kc, h, 0:65],
                                ecol[:, t, :],
                                start=(g == 0 and t == 0),
                                stop=(g == j and t == 3),
                                skip_group_check=True)
                    return pv, ecols

                def emit_epilogue(h, j, pv, ecols):
                    hp, hc = h % 2, h // 2
                    qcol = slice(512 * j, 512 * (j + 1))
                    # denominators: psum row 64 -> sbuf -> 4 partitions for a
                    # wide reciprocal -> back to a [1, 512] row -> broadcast.
                    drow = p2s.tile([128, 512], F32, tag="drow")
                    nc.scalar.copy(drow[64:65, :], pv[64:65, :])
                    dcol = p2s.tile([4, 128], F32, tag="dcol")
                    nc.gpsimd.dma_start(dcol[:], drow[64:65, :])
                    rcol = p2s.tile([4, 128], F32, tag="rcol")
                    nc.vector.reciprocal(rcol[:], dcol[:])
                    rrow = p2s.tile([1, 512], F32, tag="rrow")
                    nc.gpsimd.dma_start(rrow[:], rcol[:])
                    bc = p2s.tile([128, 512], F32, tag="bcs")
                    nc.gpsimd.partition_broadcast(bc[:], rrow[:])
                    # normalize + write attn
                    for g in range(j + 1):
                        gi = NGRP_HEAD * h + TRI[j] + g
                        ecol = ecols[g]
                        ast = p2a.tile([128, 4, 512], F32, tag="ast")
                        nc.vector.tensor_mul(
                            out=ast[:],
                            in0=ecol[:].bitcast(F32),
                            in1=bc[:, None, :].to_broadcast((128, 4, 512)))
                        nc.sync.dma_start(
                            attn_o.ap()[:, 4 * gi:4 * gi + 4, :], ast[:])
                    # y^T (normalized attn @ v, transposed)
                    if hp == 0:
                        nc.vector.tensor_mul(
                            out=yT[0:64, hc, qcol], in0=pv[0:64, :],
                            in1=bc[0:64, :])
                    else:
                        yst = p2s.tile([128, 512], F32R, tag="yst")
                        nc.vector.tensor_mul(
                            out=yst[0:64, :], in0=pv[0:64, :],
                            in1=bc[0:64, :])
                        nc.gpsimd.dma_start(yT[64:128, hc, qcol],
                                            yst[0:64, :])

                def emit_outproj(j):
                    for sc in range(4 * j, 4 * j + 4):
                        ost = p3.tile([128, D], F32, tag="ost")
                        for it in range(2):
                            ps = p3ps.tile([128, 512], F32, tag="op")
                            for jc in range(2):
                                nc.tensor.matmul(
                                    ps[:],
                                    yT[:, jc, 128 * sc:128 * (sc + 1)],
                                    wo_sb[:, jc, 512 * it:512 * (it + 1)],
                                    start=(jc == 0), stop=(jc == 1))
                            nc.scalar.copy(
                                ost[:, 512 * it:512 * (it + 1)], ps[:])
                        nc.sync.dma_start(out_o.ap()[sc], ost[:])

                pending = None
                pending_oproj = None
                for j in range(NJ):
                    for h in range(HL):
                        args = emit_kloop(h, j)
                        if pending is not None:
                            emit_epilogue(*pending)
                            if pending[0] == HL - 1:
                                pending_oproj = pending[1]
                        elif pending_oproj is not None:
                            pass
                        if pending_oproj is not None and h == 1:
                            emit_outproj(pending_oproj)
                            pending_oproj = None
                        pending = (h, j) + args
                emit_epilogue(*pending)
                emit_outproj(NJ - 1)

    nc.compile()
    return nc


def _get_nc():
    global _NC_CACHE
    if _NC_CACHE is None:
        _NC_CACHE = _build_nc()
    return _NC_CACHE


def _rope_tables():
    inv = (np.float32(1.0) /
           (np.float32(10000.0) **
            (np.arange(0, DK, 2, dtype=np.float32) / np.float32(DK))))
    pos = np.arange(S, dtype=np.float32)
    ang = pos[:, None] * inv[None, :]                    # [S, DK/2] f32
    sin = np.repeat(np.sin(ang), 2, axis=-1)             # [S, DK]
    cos = np.repeat(np.cos(ang), 2, axis=-1)
    sgn = np.where(np.arange(DK) % 2 == 0, np.float32(-1), np.float32(1))
    cosT = np.ascontiguousarray(np.tile(cos.T, (2, 1)), dtype=np.float32)
    ssT = np.ascontiguousarray(np.tile((sin * sgn).T, (2, 1)),
                               dtype=np.float32)
    return cosT, ssT  # each [128, S]


def _numpy_reference(Q, K, V, Wq, bq, Wk, bk, Wv, bv, Wo, bo, mask):
    """Fallback for non-causal masks (never hit with the standard inputs)."""
    def proj(x, W, b):
        return (x @ W.T + b).reshape(B, S, H, DK).transpose(0, 2, 1, 3)
    q, k, v = proj(Q, Wq, bq), proj(K, Wk, bk), proj(V, Wv, bv)
    inv = (np.float32(1.0) /
           (np.float32(10000.0) **
            (np.arange(0, DK, 2, dtype=np.float32) / np.float32(DK))))
    ang = np.arange(S, dtype=np.float32)[:, None] * inv[None, :]
    sin_u = np.repeat(np.sin(ang), 2, -1)
    cos_u = np.repeat(np.cos(ang), 2, -1)

    def rot(x):
        x1, x2 = x[..., 0::2], x[..., 1::2]
        return np.stack((-x2, x1), -1).reshape(x.shape)
    q = q * cos_u[None, None] + rot(q) * sin_u[None, None]
    k = k * cos_u[None, None] + rot(k) * sin_u[None, None]
    sc = np.einsum('bhqd,bhkd->bhqk', q, k) / np.sqrt(np.float32(DK))
    sc = np.where(mask == 0, -np.inf, sc)
    sc = sc - sc.max(-1, keepdims=True)
    e = np.exp(sc)
    attn = e / e.sum(-1, keepdims=True)
    out = np.einsum('bhqk,bhkd->bhqd', attn, v)
    out = out.transpose(0, 2, 1, 3).reshape(B, S, D) @ Wo.T + bo
    return out.astype(np.float32), attn.astype(np.float32)


def kernel(Q, K, V, Wq, bq, Wk, bk, Wv, bv, Wo, bo, mask):
    global LAST_RESULTS
    Q = np.asarray(Q, np.float32)
    K = np.asarray(K, np.float32)
    V = np.asarray(V, np.float32)
    Wq, Wk, Wv, Wo = (np.asarray(a, np.float32) for a in (Wq, Wk, Wv, Wo))
    bq, bk, bv, bo = (np.asarray(a, np.float32) for a in (bq, bk, bv, bo))
    mask = np.asarray(mask)

    causal = bool(
        (mask == np.tril(np.ones((S, S), mask.dtype))[None, None]).all())
    if not causal:
        return _numpy_reference(Q, K, V, Wq, bq, Wk, bk, Wv, bv, Wo, bo, mask)

    nc = _get_nc()
    cosT, ssT = _rope_tables()

    # diagonal-block 0/1 mask tiles in [k, q] layout: tile (j, t) covers
    # k in [512j+128t, +128), q in [512j, +512)
    mdiag = np.zeros((128, 4 * NJ, 512), np.float32)
    for j in range(NJ):
        for t in range(4):
            k0 = 512 * j + 128 * t
            kk = np.arange(k0, k0 + 128)
            qq = np.arange(512 * j, 512 * (j + 1))
            mdiag[:, 4 * j + t, :] = (kk[:, None] <= qq[None, :])

    in_maps = []
    for c in range(NCORES):
        b, g = c // GRP, c % GRP
        sl = slice(DL * g, DL * (g + 1))
        in_maps.append({
            "xt_q": np.ascontiguousarray(Q[b].T),
            "xt_k": np.ascontiguousarray(K[b].T),
            "xt_v": np.ascontiguousarray(V[b].T),
            "w_qT": np.ascontiguousarray(Wq[sl].T),
            "w_kT": np.ascontiguousarray(Wk[sl].T),
            "w_vT": np.ascontiguousarray(Wv[sl].T),
            "w_oT": np.ascontiguousarray(Wo[:, sl].T),
            "cos_t": cosT,
            "sin_t": ssT,
            "bq_s": np.ascontiguousarray(bq[sl].reshape(2, 128).T),
            "bk_s": np.ascontiguousarray(bk[sl].reshape(2, 128).T),
            "mask_diag": mdiag,
        })

    trace = bool(int(os.environ.get("KERNEL_TRACE", "0")))
    res = bass_utils.run_bass_kernel_spmd(
        nc, in_maps, core_ids=list(range(NCORES)), trace=trace)
    LAST_RESULTS = res

    attn = np.zeros((B, H, S, S), np.float32)
    out = np.zeros((B, S, D), np.float32)
    for c in range(NCORES):
        b, g = c // GRP, c % GRP
        chunk = res.results[c]["attn_out"].reshape(128, NBLK, 4, 512)
        for h in range(HL):
            for j in range(NJ):
                for gg in range(j + 1):
                    gi = NGRP_HEAD * h + TRI[j] + gg
                    blk = chunk[:, gi]  # [p, t, q]
                    attn[b, HL * g + h, 512 * j:512 * (j + 1),
                         512 * gg:512 * (gg + 1)] = (
                        blk.transpose(2, 1, 0).reshape(512, 512))
        out[b] += res.results[c]["out_part"].reshape(S, D)
    out += (bv @ Wo.T + bo)[None, None, :]
    return out, attn


# revision 18
# speedup vs baseline: 1.3780x; 1.0295x over previous
"""Multi-head attention (16 heads, RoPE, causal) on 8 Trainium2 NeuronCores.

Sharding: batch*heads across cores. Core c handles batch b = c//4 and heads
4*(c%4) .. 4*(c%4)+3 (column-split W_q/W_k/W_v, row-split W_o; partial
outputs summed on host).

On-device layout choices:
  - q,k produced transposed [d_head, s] so scores can be computed as
    scoresT [k, q] = kT.T @ qT (contraction over d on partitions).
  - softmax runs over the partition (k) dim: exp on ACT straight out of
    PSUM; the denominator comes free from a ones-column appended to V in
    the attn@V matmul (outT[64] = sum_k exp).
  - causal structure: for query tile j (512 wide) only k-chunks <= 4j+3
    are computed; the diagonal 4-chunk group is masked with a
    host-precomputed 0/1 tile; everything above the diagonal is skipped
    (host leaves zeros).
  - attn is written to DRAM packed/transposed; the host scatters it into
    the [B,H,S,S] output (pure data movement, no math).
Matmuls use float32r (full fp32 storage, single-pass PE) except the
exactness-sensitive broadcast outer-product which stays fp32.
"""

import os
import numpy as np

import concourse.bass as bass
import concourse.bacc as bacc
import concourse.mybir as mybir
import concourse.tile as tile
from concourse import bass_utils

F32 = mybir.dt.float32
F32R = mybir.dt.float32r

B, S, D, H, DK = 2, 2048, 1024, 16, 64
NCORES = 8
GRP = NCORES // B          # core groups per batch
HL = H // GRP              # heads per core
DL = HL * DK               # local projected dim
NJ = S // 512              # query tiles per head
NKC = S // 128             # k chunks
NDC = D // 128             # contraction chunks for projections
TRI = [0, 1, 3, 6]         # sum_{i<j}(i+1)
NGRP_HEAD = TRI[NJ - 1] + NJ          # kept 4-chunk groups per head (10)
NBLK = HL * NGRP_HEAD                 # kept groups per core (40)
SWAP_MASK = [i ^ 1 for i in range(32)]

_NC_CACHE = None
LAST_RESULTS = None  # BassKernelResults of the most recent kernel() call


def _r(ap):
    return ap.bitcast(F32R)


def _build_nc():
    nc = bacc.Bacc("TRN2", target_bir_lowering=False, debug=False,
                   enable_asserts=False, num_devices=NCORES)

    xt_q = nc.dram_tensor("xt_q", [D, S], F32R, kind="ExternalInput")
    xt_k = nc.dram_tensor("xt_k", [D, S], F32R, kind="ExternalInput")
    xt_v = nc.dram_tensor("xt_v", [D, S], F32R, kind="ExternalInput")
    w_q = nc.dram_tensor("w_qT", [D, DL], F32R, kind="ExternalInput")
    w_k = nc.dram_tensor("w_kT", [D, DL], F32R, kind="ExternalInput")
    w_v = nc.dram_tensor("w_vT", [D, DL], F32R, kind="ExternalInput")
    w_o = nc.dram_tensor("w_oT", [DL, D], F32R, kind="ExternalInput")
    cos_d = nc.dram_tensor("cos_t", [128, S], F32, kind="ExternalInput")
    sin_d = nc.dram_tensor("sin_t", [128, S], F32, kind="ExternalInput")
    bq_d = nc.dram_tensor("bq_s", [128, 2], F32, kind="ExternalInput")
    bk_d = nc.dram_tensor("bk_s", [128, 2], F32, kind="ExternalInput")
    mask_d = nc.dram_tensor("mask_diag", [128, 4 * NJ, 512], F32,
                            kind="ExternalInput")
    attn_o = nc.dram_tensor("attn_out", [128, 4 * NBLK, 512], F32,
                            kind="ExternalOutput")
    out_o = nc.dram_tensor("out_part", [S // 128, 128, D], F32,
                           kind="ExternalOutput")

    with tile.TileContext(nc) as tc:
        with (
            tc.tile_pool(name="cpool", bufs=1) as cpool,
            tc.tile_pool(name="c2pool", bufs=1) as c2pool,
        ):
            wo_sb = cpool.tile([128, 2, D], F32R, tag="wo")
            bq_sb = cpool.tile([128, 2], F32, tag="bq")
            nc.sync.dma_start(bq_sb[:], bq_d.ap())
            bk_sb = cpool.tile([128, 2], F32, tag="bk")
            nc.sync.dma_start(bk_sb[:], bk_d.ap())
            ones_sb = cpool.tile([128, 128], F32, tag="ones")
            nc.vector.memset(ones_sb[:], 1.0)

            qT = c2pool.tile([128, 2, S], F32R, tag="qT")
            kT = c2pool.tile([128, 2, S], F32R, tag="kT")
            vx = c2pool.tile([128, NKC, HL, 66], F32R, tag="vx")
            yT = c2pool.tile([128, 2, S], F32R, tag="yT")
            nc.vector.tensor_copy(
                vx[:, :, :, 64:65],
                ones_sb[:, 0:1][:, :, None, None].to_broadcast(
                    (128, NKC, HL, 1)))

            # ---------------- phase 1: projections + rope ----------------
            with (
                tc.tile_pool(name="p1w", bufs=1) as p1w,
                tc.tile_pool(name="p1x", bufs=3) as p1x,
                tc.tile_pool(name="p1r", bufs=2) as p1r,
                tc.tile_pool(name="p1ps", bufs=3, space="PSUM") as p1ps,
            ):
                wq_sb = p1w.tile([128, NDC, DL], F32R, tag="wq")
                nc.sync.dma_start(
                    wq_sb[:], w_q.ap().rearrange("(dc p) o -> p dc o", p=128))
                wk_sb = p1w.tile([128, NDC, DL], F32R, tag="wk")
                wv_sb = p1w.tile([128, NDC, DL], F32R, tag="wv")
                cos_sb = p1w.tile([128, S], F32, tag="cos")
                sin_sb = p1w.tile([128, S], F32, tag="sin")

                for st in range(S // 512):
                    scol = slice(512 * st, 512 * (st + 1))

                    for name, xdram, wsb, bsb, dst in (
                        ("q", xt_q, wq_sb, bq_sb, qT),
                        ("k", xt_k, wk_sb, bk_sb, kT),
                    ):
                        xsb = p1x.tile([128, NDC, 512], F32R, tag="xt")
                        nc.sync.dma_start(
                            xsb[:],
                            xdram.ap()[:, scol].rearrange(
                                "(dc p) s -> p dc s", p=128))
                        if st == 0 and name == "q":
                            nc.sync.dma_start(cos_sb[:], cos_d.ap())
                            nc.sync.dma_start(sin_sb[:], sin_d.ap())
                        if st == 0 and name == "k":
                            nc.sync.dma_start(
                                wk_sb[:],
                                w_k.ap().rearrange("(dc p) o -> p dc o",
                                                   p=128))
                        for dt in range(2):
                            ps = p1ps.tile([128, 512], F32, tag="qk")
                            for dc in range(NDC):
                                nc.tensor.matmul(
                                    ps[:],
                                    wsb[:, dc, 128 * dt:128 * (dt + 1)],
                                    xsb[:, dc, :],
                                    start=(dc == 0), stop=(dc == NDC - 1))
                            raw = p1r.tile([128, 512], F32, tag="raw")
                            nc.vector.tensor_scalar_add(
                                raw[:], ps[:], bsb[:, dt:dt + 1])
                            swp = p1r.tile([128, 512], F32, tag="swp")
                            nc.vector.stream_shuffle(swp[:], raw[:], SWAP_MASK)
                            tmp = p1r.tile([128, 512], F32, tag="tmp")
                            nc.vector.tensor_mul(
                                out=tmp[:], in0=swp[:], in1=sin_sb[:, scol])
                            tmp2 = p1r.tile([128, 512], F32, tag="tmp2")
                            nc.vector.tensor_mul(
                                out=tmp2[:], in0=raw[:], in1=cos_sb[:, scol])
                            nc.vector.tensor_add(
                                out=dst[:, dt, scol], in0=tmp2[:],
                                in1=tmp[:])

                    xsb = p1x.tile([128, NDC, 512], F32R, tag="xt")
                    nc.sync.dma_start(
                        xsb[:],
                        xt_v.ap()[:, scol].rearrange("(dc p) s -> p dc s",
                                                     p=128))
                    if st == 0:
                        nc.sync.dma_start(
                            wv_sb[:],
                            w_v.ap().rearrange("(dc p) o -> p dc o", p=128))
                    for si in range(4):
                        sc = 4 * st + si
                        ps = p1ps.tile([128, DL], F32, tag="v")
                        for dc in range(NDC):
                            nc.tensor.matmul(
                                ps[:],
                                xsb[:, dc, 128 * si:128 * (si + 1)],
                                wv_sb[:, dc, :],
                                start=(dc == 0), stop=(dc == NDC - 1))
                        nc.scalar.copy(
                            vx[:, sc, :, 0:64],
                            ps.rearrange("p (h d) -> p h d", d=DK))

            # ------------- phase 2+3: attention + output projection -------------
            with (
                tc.tile_pool(name="p2m", bufs=2) as p2m,
                tc.tile_pool(name="p2e", bufs=8) as p2e,
                tc.tile_pool(name="p2a", bufs=2) as p2a,
                tc.tile_pool(name="p2s", bufs=2) as p2s,
                tc.tile_pool(name="p2t", bufs=3) as p2t,
                tc.tile_pool(name="p3", bufs=2) as p3,
                tc.tile_pool(name="p2ps", bufs=2, space="PSUM") as p2ps,
                tc.tile_pool(name="p2sc", bufs=2, space="PSUM") as p2sc,
                tc.tile_pool(name="p3ps", bufs=2, space="PSUM") as p3ps,
            ):
                nc.sync.dma_start(
                    wo_sb[:], w_o.ap().rearrange("(jc p) i -> p jc i", p=128))

                masks = {}

                def emit_kloop(h, j):
                    hp, hc = h % 2, h // 2
                    prow = slice(64 * hp, 64 * hp + 64)
                    qcol = slice(512 * j, 512 * (j + 1))
                    pv = p2ps.tile([128, 512], F32, tag="pv")
                    if h == 0:
                        m = p2m.tile([128, 4, 512], F32, tag="mask")
                        nc.sync.dma_start(
                            m[:], mask_d.ap()[:, 4 * j:4 * j + 4, :])
                        masks[j] = m
                    mask_sb = masks[j]
                    ecols = []
                    for g in range(j + 1):
                        ecol = p2e.tile([128, 4, 512], F32R, tag="ecol")
                        ecols.append(ecol)
                        for u in range(2):
                            sc_ps = p2sc.tile([128, 2, 512], F32, tag="sc")
                            for w in range(2):
                                kc = 4 * g + 2 * u + w
                                nc.tensor.matmul(
                                    sc_ps[:, w, :],
                                    kT[prow, hc, 128 * kc:128 * (kc + 1)],
                                    qT[prow, hc, qcol],
                                    start=True, stop=True)
                            nc.scalar.activation(
                                ecol[:, 2 * u:2 * u + 2, :], sc_ps[:],
                                mybir.ActivationFunctionType.Exp,
                                scale=float(1.0 / np.sqrt(DK)))
                        if g == j:
                            nc.vector.tensor_mul(
                                out=ecol[:],
                                in0=ecol[:].bitcast(F32),
                                in1=mask_sb[:])
                        for t in range(4):
                            kc = 4 * g + t
                            nc.tensor.matmul(
                                pv[0:65, :],
                                vx[:, kc, h, 0:65],
                                ecol[:, t, :],
                                start=(g == 0 and t == 0),
                                stop=(g == j and t == 3),
                                skip_group_check=True)
                    return pv, ecols

                def emit_epilogue(h, j, pv, ecols):
                    hp, hc = h % 2, h // 2
                    qcol = slice(512 * j, 512 * (j + 1))
                    # denominators: psum row 64 -> sbuf -> 4 partitions for a
                    # wide reciprocal -> back to a [1, 512] row -> broadcast.
                    drow = p2s.tile([128, 512], F32, tag="drow")
                    nc.scalar.copy(drow[64:65, :], pv[64:65, :])
                    dcol = p2t.tile([4, 128], F32, tag="dcol")
                    nc.gpsimd.dma_start(dcol[:], drow[64:65, :])
                    rcol = p2t.tile([4, 128], F32, tag="rcol")
                    nc.vector.reciprocal(rcol[:], dcol[:])
                    rrow = p2t.tile([1, 512], F32, tag="rrow")
                    nc.gpsimd.dma_start(rrow[:], rcol[:])
                    bc = p2s.tile([128, 512], F32, tag="bcs")
                    nc.gpsimd.partition_broadcast(bc[:], rrow[:])
                    # normalize + write attn
                    for g in range(j + 1):
                        gi = NGRP_HEAD * h + TRI[j] + g
                        ecol = ecols[g]
                        ast = p2a.tile([128, 4, 512], F32, tag="ast")
                        nc.vector.tensor_mul(
                            out=ast[:],
                            in0=ecol[:].bitcast(F32),
                            in1=bc[:, None, :].to_broadcast((128, 4, 512)))
                        nc.sync.dma_start(
                            attn_o.ap()[:, 4 * gi:4 * gi + 4, :], ast[:])
                    # y^T (normalized attn @ v, transposed)
                    if hp == 0:
                        nc.vector.tensor_mul(
                            out=yT[0:64, hc, qcol], in0=pv[0:64, :],
                            in1=bc[0:64, :])
                    else:
                        yst = p2s.tile([128, 512], F32R, tag="yst")
                        nc.vector.tensor_mul(
                            out=yst[0:64, :], in0=pv[0:64, :],
                            in1=bc[0:64, :])
                        nc.gpsimd.dma_start(yT[64:128, hc, qcol],
                                            yst[0:64, :])

                def emit_outproj(j):
                    for sc in range(4 * j, 4 * j + 4):
                        ost = p3.tile([128, D], F32, tag="ost")
                        for it in range(2):
                            ps = p3ps.tile([128, 512], F32, tag="op")
                            for jc in range(2):
                                nc.tensor.matmul(
                                    ps[:],
                                    yT[:, jc, 128 * sc:128 * (sc + 1)],
                                    wo_sb[:, jc, 512 * it:512 * (it + 1)],
                                    start=(jc == 0), stop=(jc == 1))
                            nc.scalar.copy(
                                ost[:, 512 * it:512 * (it + 1)], ps[:])
                        nc.sync.dma_start(out_o.ap()[sc], ost[:])

                pending = None
                pending_oproj = None
                for j in range(NJ):
                    for h in range(HL):
                        args = emit_kloop(h, j)
                        if pending is not None:
                            emit_epilogue(*pending)
                            if pending[0] == HL - 1:
                                pending_oproj = pending[1]
                        elif pending_oproj is not None:
                            pass
                        if pending_oproj is not None and h == 1:
                            emit_outproj(pending_oproj)
                            pending_oproj = None
                        pending = (h, j) + args
                emit_epilogue(*pending)
                emit_outproj(NJ - 1)

    nc.compile()
    return nc


def _get_nc():
    global _NC_CACHE
    if _NC_CACHE is None:
        _NC_CACHE = _build_nc()
    return _NC_CACHE


def _rope_tables():
    inv = (np.float32(1.0) /
           (np.float32(10000.0) **
            (np.arange(0, DK, 2, dtype=np.float32) / np.float32(DK))))
    pos = np.arange(S, dtype=np.float32)
    ang = pos[:, None] * inv[None, :]                    # [S, DK/2] f32
    sin = np.repeat(np.sin(ang), 2, axis=-1)             # [S, DK]
    cos = np.repeat(np.cos(ang), 2, axis=-1)
    sgn = np.where(np.arange(DK) % 2 == 0, np.float32(-1), np.float32(1))
    cosT = np.ascontiguousarray(np.tile(cos.T, (2, 1)), dtype=np.float32)
    ssT = np.ascontiguousarray(np.tile((sin * sgn).T, (2, 1)),
                               dtype=np.float32)
    return cosT, ssT  # each [128, S]


def _numpy_reference(Q, K, V, Wq, bq, Wk, bk, Wv, bv, Wo, bo, mask):
    """Fallback for non-causal masks (never hit with the standard inputs)."""
    def proj(x, W, b):
        return (x @ W.T + b).reshape(B, S, H, DK).transpose(0, 2, 1, 3)
    q, k, v = proj(Q, Wq, bq), proj(K, Wk, bk), proj(V, Wv, bv)
    inv = (np.float32(1.0) /
           (np.float32(10000.0) **
            (np.arange(0, DK, 2, dtype=np.float32) / np.float32(DK))))
    ang = np.arange(S, dtype=np.float32)[:, None] * inv[None, :]
    sin_u = np.repeat(np.sin(ang), 2, -1)
    cos_u = np.repeat(np.cos(ang), 2, -1)

    def rot(x):
        x1, x2 = x[..., 0::2], x[..., 1::2]
        return np.stack((-x2, x1), -1).reshape(x.shape)
    q = q * cos_u[None, None] + rot(q) * sin_u[None, None]
    k = k * cos_u[None, None] + rot(k) * sin_u[None, None]
    sc = np.einsum('bhqd,bhkd->bhqk', q, k) / np.sqrt(np.float32(DK))
    sc = np.where(mask == 0, -np.inf, sc)
    sc = sc - sc.max(-1, keepdims=True)
    e = np.exp(sc)
    attn = e / e.sum(-1, keepdims=True)
    out = np.einsum('bhqk,bhkd->bhqd', attn, v)
    out = out.transpose(0, 2, 1, 3).reshape(B, S, D) @ Wo.T + bo
    return out.astype(np.float32), attn.astype(np.float32)


def kernel(Q, K, V, Wq, bq, Wk, bk, Wv, bv, Wo, bo, mask):
    global LAST_RESULTS
    Q = np.asarray(Q, np.float32)
    K = np.asarray(K, np.float32)
    V = np.asarray(V, np.float32)
    Wq, Wk, Wv, Wo = (np.asarray(a, np.float32) for a in (Wq, Wk, Wv, Wo))
    bq, bk, bv, bo = (np.asarray(a, np.float32) for a in (bq, bk, bv, bo))
    mask = np.asarray(mask)

    causal = bool(
        (mask == np.tril(np.ones((S, S), mask.dtype))[None, None]).all())
    if not causal:
        return _numpy_reference(Q, K, V, Wq, bq, Wk, bk, Wv, bv, Wo, bo, mask)

    nc = _get_nc()
    cosT, ssT = _rope_tables()

    # diagonal-block 0/1 mask tiles in [k, q] layout: tile (j, t) covers
    # k in [512j+128t, +128), q in [512j, +512)
    mdiag = np.zeros((128, 4 * NJ, 512), np.float32)
    for j in range(NJ):
        for t in range(4):
            k0 = 512 * j + 128 * t
            kk = np.arange(k0, k0 + 128)
            qq = np.arange(512 * j, 512 * (j + 1))
            mdiag[:, 4 * j + t, :] = (kk[:, None] <= qq[None, :])

    in_maps = []
    for c in range(NCORES):
        b, g = c // GRP, c % GRP
        sl = slice(DL * g, DL * (g + 1))
        in_maps.append({
            "xt_q": np.ascontiguousarray(Q[b].T),
            "xt_k": np.ascontiguousarray(K[b].T),
            "xt_v": np.ascontiguousarray(V[b].T),
            "w_qT": np.ascontiguousarray(Wq[sl].T),
            "w_kT": np.ascontiguousarray(Wk[sl].T),
            "w_vT": np.ascontiguousarray(Wv[sl].T),
            "w_oT": np.ascontiguousarray(Wo[:, sl].T),
            "cos_t": cosT,
            "sin_t": ssT,
            "bq_s": np.ascontiguousarray(bq[sl].reshape(2, 128).T),
            "bk_s": np.ascontiguousarray(bk[sl].reshape(2, 128).T),
            "mask_diag": mdiag,
        })

    trace = bool(int(os.environ.get("KERNEL_TRACE", "0")))
    res = bass_utils.run_bass_kernel_spmd(
        nc, in_maps, core_ids=list(range(NCORES)), trace=trace)
    LAST_RESULTS = res

    attn = np.zeros((B, H, S, S), np.float32)
    out = np.zeros((B, S, D), np.float32)
    for c in range(NCORES):
        b, g = c // GRP, c % GRP
        chunk = res.results[c]["attn_out"].reshape(128, NBLK, 4, 512)
        for h in range(HL):
            for j in range(NJ):
                for gg in range(j + 1):
                    gi = NGRP_HEAD * h + TRI[j] + gg
                    blk = chunk[:, gi]  # [p, t, q]
                    attn[b, HL * g + h, 512 * j:512 * (j + 1),
                         512 * gg:512 * (gg + 1)] = (
                        blk.transpose(2, 1, 0).reshape(512, 512))
        out[b] += res.results[c]["out_part"].reshape(S, D)
    out += (bv @ Wo.T + bo)[None, None, :]
    return out, attn


# revision 19
# speedup vs baseline: 1.4756x; 1.0709x over previous
"""Multi-head attention (16 heads, RoPE, causal) on 8 Trainium2 NeuronCores.

Sharding: batch*heads across cores. Core c handles batch b = c//4 and heads
4*(c%4) .. 4*(c%4)+3 (column-split W_q/W_k/W_v, row-split W_o; partial
outputs summed on host).

On-device layout choices:
  - q,k produced transposed [d_head, s] so scores can be computed as
    scoresT [k, q] = kT.T @ qT (contraction over d on partitions).
  - softmax runs over the partition (k) dim: exp on ACT straight out of
    PSUM; the denominator comes free from a ones-column appended to V in
    the attn@V matmul (outT[64] = sum_k exp).
  - causal structure: for query tile j (512 wide) only k-chunks <= 4j+3
    are computed; the diagonal 4-chunk group is masked with a
    host-precomputed 0/1 tile; everything above the diagonal is skipped
    (host leaves zeros).
  - attn is written to DRAM packed/transposed; the host scatters it into
    the [B,H,S,S] output (pure data movement, no math).
Matmuls use float32r (full fp32 storage, single-pass PE) except the
exactness-sensitive broadcast outer-product which stays fp32.
"""

import os
import numpy as np

import concourse.bass as bass
import concourse.bacc as bacc
import concourse.mybir as mybir
import concourse.tile as tile
from concourse import bass_utils

F32 = mybir.dt.float32
F32R = mybir.dt.float32r

B, S, D, H, DK = 2, 2048, 1024, 16, 64
NCORES = 8
GRP = NCORES // B          # core groups per batch
HL = H // GRP              # heads per core
DL = HL * DK               # local projected dim
NJ = S // 512              # query tiles per head
NKC = S // 128             # k chunks
NDC = D // 128             # contraction chunks for projections
TRI = [0, 1, 3, 6]         # sum_{i<j}(i+1)
NGRP_HEAD = TRI[NJ - 1] + NJ          # kept 4-chunk groups per head (10)
NBLK = HL * NGRP_HEAD                 # kept groups per core (40)
SWAP_MASK = [i ^ 1 for i in range(32)]

_NC_CACHE = None
LAST_RESULTS = None  # BassKernelResults of the most recent kernel() call


def _r(ap):
    return ap.bitcast(F32R)


def _build_nc():
    nc = bacc.Bacc("TRN2", target_bir_lowering=False, debug=False,
                   enable_asserts=False, num_devices=NCORES)

    xt_q = nc.dram_tensor("xt_q", [D, S], F32R, kind="ExternalInput")
    xt_k = nc.dram_tensor("xt_k", [D, S], F32R, kind="ExternalInput")
    xt_v = nc.dram_tensor("xt_v", [D, S], F32R, kind="ExternalInput")
    w_q = nc.dram_tensor("w_qT", [D, DL], F32R, kind="ExternalInput")
    w_k = nc.dram_tensor("w_kT", [D, DL], F32R, kind="ExternalInput")
    w_v = nc.dram_tensor("w_vT", [D, DL], F32R, kind="ExternalInput")
    w_o = nc.dram_tensor("w_oT", [DL, D], F32R, kind="ExternalInput")
    cos_d = nc.dram_tensor("cos_t", [128, S], F32, kind="ExternalInput")
    sin_d = nc.dram_tensor("sin_t", [128, S], F32, kind="ExternalInput")
    bq_d = nc.dram_tensor("bq_s", [128, 2], F32, kind="ExternalInput")
    bk_d = nc.dram_tensor("bk_s", [128, 2], F32, kind="ExternalInput")
    mask_d = nc.dram_tensor("mask_diag", [128, 4 * NJ, 512], F32,
                            kind="ExternalInput")
    attn_o = nc.dram_tensor("attn_out", [128, 4 * NBLK, 512], F32,
                            kind="ExternalOutput")
    out_o = nc.dram_tensor("out_part", [S // 128, 128, D], F32,
                           kind="ExternalOutput")

    with tile.TileContext(nc) as tc:
        with (
            tc.tile_pool(name="cpool", bufs=1) as cpool,
            tc.tile_pool(name="c2pool", bufs=1) as c2pool,
        ):
            wo_sb = cpool.tile([128, 2, D], F32R, tag="wo")
            bq_sb = cpool.tile([128, 2], F32, tag="bq")
            nc.sync.dma_start(bq_sb[:], bq_d.ap())
            bk_sb = cpool.tile([128, 2], F32, tag="bk")
            nc.sync.dma_start(bk_sb[:], bk_d.ap())
            ones_sb = cpool.tile([128, 128], F32, tag="ones")
            nc.vector.memset(ones_sb[:], 1.0)

            qT = c2pool.tile([128, 2, S], F32R, tag="qT")
            kT = c2pool.tile([128, 2, S], F32R, tag="kT")
            vx = c2pool.tile([128, NKC, HL, 66], F32R, tag="vx")
            yT = c2pool.tile([128, 2, S], F32R, tag="yT")
            nc.vector.tensor_copy(
                vx[:, :, :, 64:65],
                ones_sb[:, 0:1][:, :, None, None].to_broadcast(
                    (128, NKC, HL, 1)))

            # ---------------- phase 1: projections + rope ----------------
            with (
                tc.tile_pool(name="p1w", bufs=1) as p1w,
                tc.tile_pool(name="p1x", bufs=3) as p1x,
                tc.tile_pool(name="p1r", bufs=2) as p1r,
                tc.tile_pool(name="p1ps", bufs=3, space="PSUM") as p1ps,
            ):
                wq_sb = p1w.tile([128, NDC, DL], F32R, tag="wq")
                nc.sync.dma_start(
                    wq_sb[:], w_q.ap().rearrange("(dc p) o -> p dc o", p=128))
                wk_sb = p1w.tile([128, NDC, DL], F32R, tag="wk")
                wv_sb = p1w.tile([128, NDC, DL], F32R, tag="wv")
                cos_sb = p1w.tile([128, S], F32, tag="cos")
                sin_sb = p1w.tile([128, S], F32, tag="sin")

                for st in range(S // 512):
                    scol = slice(512 * st, 512 * (st + 1))

                    for name, xdram, wsb, bsb, dst in (
                        ("q", xt_q, wq_sb, bq_sb, qT),
                        ("k", xt_k, wk_sb, bk_sb, kT),
                    ):
                        xsb = p1x.tile([128, NDC, 512], F32R, tag="xt")
                        nc.sync.dma_start(
                            xsb[:],
                            xdram.ap()[:, scol].rearrange(
                                "(dc p) s -> p dc s", p=128))
                        if st == 0 and name == "q":
                            nc.sync.dma_start(cos_sb[:], cos_d.ap())
                            nc.sync.dma_start(sin_sb[:], sin_d.ap())
                        if st == 0 and name == "k":
                            nc.sync.dma_start(
                                wk_sb[:],
                                w_k.ap().rearrange("(dc p) o -> p dc o",
                                                   p=128))
                        for dt in range(2):
                            ps = p1ps.tile([128, 512], F32, tag="qk")
                            for dc in range(NDC):
                                nc.tensor.matmul(
                                    ps[:],
                                    wsb[:, dc, 128 * dt:128 * (dt + 1)],
                                    xsb[:, dc, :],
                                    start=(dc == 0), stop=(dc == NDC - 1))
                            raw = p1r.tile([128, 512], F32, tag="raw")
                            nc.scalar.activation(
                                raw[:], ps[:],
                                mybir.ActivationFunctionType.Identity,
                                bias=bsb[:, dt:dt + 1])
                            swp = p1r.tile([128, 512], F32, tag="swp")
                            nc.vector.stream_shuffle(swp[:], raw[:], SWAP_MASK)
                            tmp = p1r.tile([128, 512], F32, tag="tmp")
                            nc.vector.tensor_mul(
                                out=tmp[:], in0=swp[:], in1=sin_sb[:, scol])
                            tmp2 = p1r.tile([128, 512], F32, tag="tmp2")
                            nc.vector.tensor_mul(
                                out=tmp2[:], in0=raw[:], in1=cos_sb[:, scol])
                            nc.gpsimd.tensor_add(
                                out=dst[:, dt, scol], in0=tmp2[:],
                                in1=tmp[:])

                    xsb = p1x.tile([128, NDC, 512], F32R, tag="xt")
                    nc.sync.dma_start(
                        xsb[:],
                        xt_v.ap()[:, scol].rearrange("(dc p) s -> p dc s",
                                                     p=128))
                    if st == 0:
                        nc.sync.dma_start(
                            wv_sb[:],
                            w_v.ap().rearrange("(dc p) o -> p dc o", p=128))
                    for si in range(4):
                        sc = 4 * st + si
                        ps = p1ps.tile([128, DL], F32, tag="v")
                        for dc in range(NDC):
                            nc.tensor.matmul(
                                ps[:],
                                xsb[:, dc, 128 * si:128 * (si + 1)],
                                wv_sb[:, dc, :],
                                start=(dc == 0), stop=(dc == NDC - 1))
                        nc.scalar.copy(
                            vx[:, sc, :, 0:64],
                            ps.rearrange("p (h d) -> p h d", d=DK))

            # ------------- phase 2+3: attention + output projection -------------
            with (
                tc.tile_pool(name="p2m", bufs=2) as p2m,
                tc.tile_pool(name="p2e", bufs=8) as p2e,
                tc.tile_pool(name="p2a", bufs=2) as p2a,
                tc.tile_pool(name="p2s", bufs=2) as p2s,
                tc.tile_pool(name="p2t", bufs=3) as p2t,
                tc.tile_pool(name="p3", bufs=2) as p3,
                tc.tile_pool(name="p2ps", bufs=2, space="PSUM") as p2ps,
                tc.tile_pool(name="p2sc", bufs=2, space="PSUM") as p2sc,
                tc.tile_pool(name="p3ps", bufs=2, space="PSUM") as p3ps,
            ):
                nc.sync.dma_start(
                    wo_sb[:], w_o.ap().rearrange("(jc p) i -> p jc i", p=128))

                masks = {}

                def emit_kloop(h, j):
                    hp, hc = h % 2, h // 2
                    prow = slice(64 * hp, 64 * hp + 64)
                    qcol = slice(512 * j, 512 * (j + 1))
                    pv = p2ps.tile([128, 512], F32, tag="pv")
                    if h == 0:
                        m = p2m.tile([128, 4, 512], F32, tag="mask")
                        nc.sync.dma_start(
                            m[:], mask_d.ap()[:, 4 * j:4 * j + 4, :])
                        masks[j] = m
                    mask_sb = masks[j]
                    ecols = []
                    for g in range(j + 1):
                        ecol = p2e.tile([128, 4, 512], F32R, tag="ecol")
                        ecols.append(ecol)
                        for u in range(2):
                            sc_ps = p2sc.tile([128, 2, 512], F32, tag="sc")
                            for w in range(2):
                                kc = 4 * g + 2 * u + w
                                nc.tensor.matmul(
                                    sc_ps[:, w, :],
                                    kT[prow, hc, 128 * kc:128 * (kc + 1)],
                                    qT[prow, hc, qcol],
                                    start=True, stop=True)
                            nc.scalar.activation(
                                ecol[:, 2 * u:2 * u + 2, :], sc_ps[:],
                                mybir.ActivationFunctionType.Exp,
                                scale=float(1.0 / np.sqrt(DK)))
                        if g == j:
                            nc.vector.tensor_mul(
                                out=ecol[:],
                                in0=ecol[:].bitcast(F32),
                                in1=mask_sb[:])
                        for t in range(4):
                            kc = 4 * g + t
                            nc.tensor.matmul(
                                pv[0:65, :],
                                vx[:, kc, h, 0:65],
                                ecol[:, t, :],
                                start=(g == 0 and t == 0),
                                stop=(g == j and t == 3),
                                skip_group_check=True)
                    return pv, ecols

                def emit_epilogue(h, j, pv, ecols):
                    hp, hc = h % 2, h // 2
                    qcol = slice(512 * j, 512 * (j + 1))
                    # denominators: psum row 64 -> sbuf -> 4 partitions for a
                    # wide reciprocal -> back to a [1, 512] row -> broadcast.
                    drow = p2s.tile([128, 512], F32, tag="drow")
                    nc.scalar.copy(drow[64:65, :], pv[64:65, :])
                    dcol = p2t.tile([4, 128], F32, tag="dcol")
                    nc.gpsimd.dma_start(dcol[:], drow[64:65, :])
                    rcol = p2t.tile([4, 128], F32, tag="rcol")
                    rscr = p2t.tile([4, 128], F32, tag="rscr")
                    nc.vector.reciprocal_approx_accurate(
                        rcol[:], dcol[:], rscr[:])
                    rrow = p2t.tile([1, 512], F32, tag="rrow")
                    nc.gpsimd.dma_start(rrow[:], rcol[:])
                    bc = p2s.tile([128, 512], F32, tag="bcs")
                    nc.gpsimd.partition_broadcast(bc[:], rrow[:])
                    # normalize + write attn
                    for g in range(j + 1):
                        gi = NGRP_HEAD * h + TRI[j] + g
                        ecol = ecols[g]
                        ast = p2a.tile([128, 4, 512], F32, tag="ast")
                        if g == j:
                            for t in range(4):
                                nc.vector.tensor_mul(
                                    out=ast[:, t, 128 * t:],
                                    in0=ecol[:, t, 128 * t:].bitcast(F32),
                                    in1=bc[:, 128 * t:])
                        else:
                            nc.vector.tensor_mul(
                                out=ast[:],
                                in0=ecol[:].bitcast(F32),
                                in1=bc[:, None, :].to_broadcast((128, 4, 512)))
                        nc.sync.dma_start(
                            attn_o.ap()[:, 4 * gi:4 * gi + 4, :], ast[:])
                    # y^T (normalized attn @ v, transposed)
                    if hp == 0:
                        nc.vector.tensor_mul(
                            out=yT[0:64, hc, qcol], in0=pv[0:64, :],
                            in1=bc[0:64, :])
                    else:
                        yst = p2s.tile([128, 512], F32R, tag="yst")
                        nc.vector.tensor_mul(
                            out=yst[0:64, :], in0=pv[0:64, :],
                            in1=bc[0:64, :])
                        nc.gpsimd.dma_start(yT[64:128, hc, qcol],
                                            yst[0:64, :])

                def emit_outproj(j):
                    for sc in range(4 * j, 4 * j + 4):
                        ost = p3.tile([128, D], F32, tag="ost")
                        for it in range(2):
                            ps = p3ps.tile([128, 512], F32, tag="op")
                            for jc in range(2):
                                nc.tensor.matmul(
                                    ps[:],
                                    yT[:, jc, 128 * sc:128 * (sc + 1)],
                                    wo_sb[:, jc, 512 * it:512 * (it + 1)],
                                    start=(jc == 0), stop=(jc == 1))
                            nc.scalar.copy(
                                ost[:, 512 * it:512 * (it + 1)], ps[:])
                        nc.sync.dma_start(out_o.ap()[sc], ost[:])

                pending = None
                pending_oproj = None
                for j in range(NJ):
                    for h in range(HL):
                        args = emit_kloop(h, j)
                        if pending is not None:
                            emit_epilogue(*pending)
                            if pending[0] == HL - 1:
                                pending_oproj = pending[1]
                        elif pending_oproj is not None:
                            pass
                        if pending_oproj is not None and h == 1:
                            emit_outproj(pending_oproj)
                            pending_oproj = None
                        pending = (h, j) + args
                emit_epilogue(*pending)
                emit_outproj(NJ - 1)

    nc.compile()
    return nc


def _get_nc():
    global _NC_CACHE
    if _NC_CACHE is None:
        _NC_CACHE = _build_nc()
    return _NC_CACHE


def _rope_tables():
    inv = (np.float32(1.0) /
           (np.float32(10000.0) **
            (np.arange(0, DK, 2, dtype=np.float32) / np.float32(DK))))
    pos = np.arange(S, dtype=np.float32)
    ang = pos[:, None] * inv[None, :]                    # [S, DK/2] f32
    sin = np.repeat(np.sin(ang), 2, axis=-1)             # [S, DK]
    cos = np.repeat(np.cos(ang), 2, axis=-1)
    sgn = np.where(np.arange(DK) % 2 == 0, np.float32(-1), np.float32(1))
    cosT = np.ascontiguousarray(np.tile(cos.T, (2, 1)), dtype=np.float32)
    ssT = np.ascontiguousarray(np.tile((sin * sgn).T, (2, 1)),
                               dtype=np.float32)
    return cosT, ssT  # each [128, S]


def _numpy_reference(Q, K, V, Wq, bq, Wk, bk, Wv, bv, Wo, bo, mask):
    """Fallback for non-causal masks (never hit with the standard inputs)."""
    def proj(x, W, b):
        return (x @ W.T + b).reshape(B, S, H, DK).transpose(0, 2, 1, 3)
    q, k, v = proj(Q, Wq, bq), proj(K, Wk, bk), proj(V, Wv, bv)
    inv = (np.float32(1.0) /
           (np.float32(10000.0) **
            (np.arange(0, DK, 2, dtype=np.float32) / np.float32(DK))))
    ang = np.arange(S, dtype=np.float32)[:, None] * inv[None, :]
    sin_u = np.repeat(np.sin(ang), 2, -1)
    cos_u = np.repeat(np.cos(ang), 2, -1)

    def rot(x):
        x1, x2 = x[..., 0::2], x[..., 1::2]
        return np.stack((-x2, x1), -1).reshape(x.shape)
    q = q * cos_u[None, None] + rot(q) * sin_u[None, None]
    k = k * cos_u[None, None] + rot(k) * sin_u[None, None]
    sc = np.einsum('bhqd,bhkd->bhqk', q, k) / np.sqrt(np.float32(DK))
    sc = np.where(mask == 0, -np.inf, sc)
    sc = sc - sc.max(-1, keepdims=True)
    e = np.exp(sc)
    attn = e / e.sum(-1, keepdims=True)
    out = np.einsum('bhqk,bhkd->bhqd', attn, v)
    out = out.transpose(0, 2, 1, 3).reshape(B, S, D) @ Wo.T + bo
    return out.astype(np.float32), attn.astype(np.float32)


def kernel(Q, K, V, Wq, bq, Wk, bk, Wv, bv, Wo, bo, mask):
    global LAST_RESULTS
    Q = np.asarray(Q, np.float32)
    K = np.asarray(K, np.float32)
    V = np.asarray(V, np.float32)
    Wq, Wk, Wv, Wo = (np.asarray(a, np.float32) for a in (Wq, Wk, Wv, Wo))
    bq, bk, bv, bo = (np.asarray(a, np.float32) for a in (bq, bk, bv, bo))
    mask = np.asarray(mask)

    causal = bool(
        (mask == np.tril(np.ones((S, S), mask.dtype))[None, None]).all())
    if not causal:
        return _numpy_reference(Q, K, V, Wq, bq, Wk, bk, Wv, bv, Wo, bo, mask)

    nc = _get_nc()
    cosT, ssT = _rope_tables()

    # diagonal-block 0/1 mask tiles in [k, q] layout: tile (j, t) covers
    # k in [512j+128t, +128), q in [512j, +512)
    mdiag = np.zeros((128, 4 * NJ, 512), np.float32)
    for j in range(NJ):
        for t in range(4):
            k0 = 512 * j + 128 * t
            kk = np.arange(k0, k0 + 128)
            qq = np.arange(512 * j, 512 * (j + 1))
            mdiag[:, 4 * j + t, :] = (kk[:, None] <= qq[None, :])

    in_maps = []
    for c in range(NCORES):
        b, g = c // GRP, c % GRP
        sl = slice(DL * g, DL * (g + 1))
        in_maps.append({
            "xt_q": np.ascontiguousarray(Q[b].T),
            "xt_k": np.ascontiguousarray(K[b].T),
            "xt_v": np.ascontiguousarray(V[b].T),
            "w_qT": np.ascontiguousarray(Wq[sl].T),
            "w_kT": np.ascontiguousarray(Wk[sl].T),
            "w_vT": np.ascontiguousarray(Wv[sl].T),
            "w_oT": np.ascontiguousarray(Wo[:, sl].T),
            "cos_t": cosT,
            "sin_t": ssT,
            "bq_s": np.ascontiguousarray(bq[sl].reshape(2, 128).T),
            "bk_s": np.ascontiguousarray(bk[sl].reshape(2, 128).T),
            "mask_diag": mdiag,
        })

    trace = bool(int(os.environ.get("KERNEL_TRACE", "0")))
    res = bass_utils.run_bass_kernel_spmd(
        nc, in_maps, core_ids=list(range(NCORES)), trace=trace)
    LAST_RESULTS = res

    attn = np.zeros((B, H, S, S), np.float32)
    out = np.zeros((B, S, D), np.float32)
    for c in range(NCORES):
        b, g = c // GRP, c % GRP
        chunk = res.results[c]["attn_out"].reshape(128, NBLK, 4, 512)
        for h in range(HL):
            for j in range(NJ):
                for gg in range(j + 1):
                    gi = NGRP_HEAD * h + TRI[j] + gg
                    blk = chunk[:, gi]  # [p, t, q]
                    blk = blk.transpose(2, 1, 0).reshape(512, 512)
                    if gg == j:
                        blk = np.tril(blk)
                    attn[b, HL * g + h, 512 * j:512 * (j + 1),
                         512 * gg:512 * (gg + 1)] = blk
        out[b] += res.results[c]["out_part"].reshape(S, D)
    out += (bv @ Wo.T + bo)[None, None, :]
    return out, attn
